# revision 11
# baseline (speedup 1.0000x reference)
"""AttackHead kernel for 8 trn2 NeuronCores.

Strategy (edges sharded data-parallel across 8 cores, everything else
replicated — no collectives):
  - Host: cast node_embeddings to bf16 and build two "wrapped" gather
    tables (row (n+32768)%65536 holds node n, so int16 indices with
    sign-extension address all 50000 rows):
      * src table row: [emb(256) | army-mask(64) in {0,-1} | pad(64)] bf16
      * tgt table row: [emb(256)] bf16
  - Device, per 2048-edge block: two dma_gather(transpose=True) calls pull
    src/tgt rows FEATURE-MAJOR into SBUF ([128 part, elem/128, 2048]).
    Per 512-edge sub-chunk: PE does h = W1.T@e (8 mm), g = A1.T@e (4 mm),
    z = W2.T@relu(h) (2 mm), army = A2.T@relu(g) (1 mm), all bf16 inputs
    with f32 PSUM accumulation. ACT applies bias+relu. DVE applies
    army mask (gathered {0,-1} * 1e9 + a2) and stages outputs.
  - Outputs are written transposed ([64, E] army, row-chunked edge logits);
    host reassembles.
"""

import numpy as np
import ml_dtypes

import concourse.bass as bass
import concourse.bacc as bacc
import concourse.mybir as mybir
import concourse.tile as tile
from concourse.library_config import mlp
from concourse.bass_utils import run_bass_kernel_spmd

BF16 = ml_dtypes.bfloat16

N_CORES = 8
E_TOTAL = 500000
N_NODES = 50000
D = 256
A_DIM = 64
import os as _os

NI = int(_os.environ.get("ATTACK_NI", 512))    # slots per gather block (HW max)
STAGE = int(_os.environ.get("ATTACK_STAGE", 4))  # debug: 1=gather,2=+hg,3=+z,4=all
ESUB = 512             # edges per matmul sub-chunk
EC = E_TOTAL // N_CORES            # 62500 edges per core
# The SWDGE gather drops TRAILING int16-negative indices, so the last slot
# of every gather block must hold a small dummy index -> 511 real edges
# per 512-slot block.
EB = NI - 1                        # real edges per block
NBLK = int(_os.environ.get("ATTACK_NBLK", (EC + EB - 1) // EB))  # 123
EPAD = NBLK * NI                   # padded slot count
WRAP = 65536

_NC_CACHE = {}


def _split_multiwaits(nc):
    """Walrus build in this container only allows 1 sync wait per
    instruction (2 for EventSemaphore); split extras onto NoOps."""
    import bass_rust

    n_fixed = 0
    for fn in nc.m.functions:
        for b in fn.blocks:
            new_list = []
            changed = False
            for ins in b.instructions:
                si = ins.sync_info
                cap = 2 if isinstance(ins, mybir.InstEventSemaphore) else 1
                if si is not None and si.on_wait and len(si.on_wait) > cap:
                    waits = list(si.on_wait)
                    for j, w in enumerate(waits[:-1]):
                        nop = mybir.InstNoOp(
                            name=f"{ins.name}_waitfix{j}", ins=[], outs=[]
                        )
                        nop.engine = ins.engine
                        nop.sync_info = bass_rust.SyncInfo(
                            on_wait=[w], on_update=[]
                        )
                        new_list.append(nop)
                    si.on_wait = [waits[-1]]
                    ins.sync_info = si
                    changed = True
                    n_fixed += 1
                new_list.append(ins)
            if changed:
                b.instructions = new_list
    return n_fixed


def _build_nc():
    f32 = mybir.dt.float32
    bf16 = mybir.dt.bfloat16
    i16 = mybir.dt.int16
    Relu = mybir.ActivationFunctionType.Relu
    Ident = mybir.ActivationFunctionType.Identity

    nc = bacc.Bacc("TRN2", target_bir_lowering=False, debug=False)
    wsrc = nc.declare_dram_parameter("wsrc", [WRAP, 384], bf16, isOutput=False)
    wtgt = nc.declare_dram_parameter("wtgt", [WRAP, 256], bf16, isOutput=False)
    sidx = nc.declare_dram_parameter("sidx", [128, EPAD // 16], i16, isOutput=False)
    tidx = nc.declare_dram_parameter("tidx", [128, EPAD // 16], i16, isOutput=False)
    w1 = nc.declare_dram_parameter("w1", [512, 256], bf16, isOutput=False)
    a1w = nc.declare_dram_parameter("a1w", [512, 128], bf16, isOutput=False)
    a2w = nc.declare_dram_parameter("a2w", [128, 64], bf16, isOutput=False)
    w2 = nc.declare_dram_parameter("w2", [256, 1], bf16, isOutput=False)
    b1r = nc.declare_dram_parameter("b1r", [128, 2], f32, isOutput=False)
    a1r = nc.declare_dram_parameter("a1r", [128, 1], f32, isOutput=False)
    b2r = nc.declare_dram_parameter("b2r", [1, 1], f32, isOutput=False)
    a2r = nc.declare_dram_parameter("a2r", [64, 1], f32, isOutput=False)
    zout = nc.declare_dram_parameter("zout", [NBLK, NI], f32, isOutput=True)
    aout = nc.declare_dram_parameter("aout", [64, EPAD], f32, isOutput=True)

    with tile.TileContext(nc) as tc:
        nc.gpsimd.load_library(mlp)
        with (
            tc.tile_pool(name="const", bufs=1) as cpool,
            tc.tile_pool(name="gather", bufs=2) as gpool,
            tc.tile_pool(name="acts", bufs=3) as hpool,
            tc.tile_pool(name="stage", bufs=2) as spool,
            tc.tile_pool(name="psumHG", bufs=2, space="PSUM") as pp,
            tc.tile_pool(name="psumZA", bufs=1, space="PSUM") as pzp,
        ):
            w1sb = cpool.tile([128, 4, 256], bf16)
            nc.sync.dma_start(
                out=w1sb[:], in_=w1[:].rearrange("(k p) m -> p k m", p=128)
            )
            a1sb = cpool.tile([128, 4, 128], bf16)
            nc.sync.dma_start(
                out=a1sb[:], in_=a1w[:].rearrange("(k p) m -> p k m", p=128)
            )
            a2sb = cpool.tile([128, 64], bf16)
            nc.sync.dma_start(out=a2sb[:], in_=a2w[:])
            w2sb = cpool.tile([128, 2, 1], bf16)
            nc.sync.dma_start(
                out=w2sb[:], in_=w2[:].rearrange("(k p) m -> p k m", p=128)
            )
            b1sb = cpool.tile([128, 2], f32)
            nc.sync.dma_start(out=b1sb[:], in_=b1r[:])
            a1bs = cpool.tile([128, 1], f32)
            nc.sync.dma_start(out=a1bs[:], in_=a1r[:])
            b2sb = cpool.tile([1, 1], f32)
            nc.sync.dma_start(out=b2sb[:], in_=b2r[:])
            a2bs = cpool.tile([64, 1], f32)
            nc.sync.dma_start(out=a2bs[:], in_=a2r[:])
            sidx_sb = cpool.tile([128, EPAD // 16], i16)
            nc.sync.dma_start(out=sidx_sb[:], in_=sidx[:])
            tidx_sb = cpool.tile([128, EPAD // 16], i16)
            nc.sync.dma_start(out=tidx_sb[:], in_=tidx[:])

            IC = NI // 16  # idx columns per block
            for J in range(NBLK):
                gs = gpool.tile([128, 3, NI], bf16, tag="gs")
                nc.gpsimd.dma_gather(
                    out_ap=gs[:],
                    in_ap=wsrc[32768:, :],
                    idxs_ap=sidx_sb[:, J * IC : (J + 1) * IC],
                    num_idxs=NI,
                    num_idxs_reg=NI,
                    elem_size=384,
                    transpose=True,
                )
                gt = gpool.tile([128, 2, NI], bf16, tag="gt")
                nc.gpsimd.dma_gather(
                    out_ap=gt[:],
                    in_ap=wtgt[32768:, :],
                    idxs_ap=tidx_sb[:, J * IC : (J + 1) * IC],
                    num_idxs=NI,
                    num_idxs_reg=NI,
                    elem_size=256,
                    transpose=True,
                )
                ast = spool.tile([64, NI], f32, tag="ast")
                zst = spool.tile([1, NI], f32, tag="zst")
                if STAGE <= 1:
                    # debug: dump gathered src chunk0 into aout rows
                    nc.vector.tensor_copy(ast[:], gs[0:64, 0, :])
                    nc.vector.tensor_copy(zst[:], gt[0:1, 0, :])
                    nc.sync.dma_start(
                        out=aout[:, J * NI : (J + 1) * NI], in_=ast[:]
                    )
                    nc.sync.dma_start(out=zout[J : J + 1, :], in_=zst[:])
                    continue
                for s in range(NI // ESUB):
                    sl = slice(s * ESUB, (s + 1) * ESUB)
                    ph0 = pp.tile([128, ESUB], f32, tag="ph0")
                    ph1 = pp.tile([128, ESUB], f32, tag="ph1")
                    pg = pp.tile([128, ESUB], f32, tag="pg")
                    rhs = [gs[:, 0, sl], gs[:, 1, sl], gt[:, 0, sl], gt[:, 1, sl]]
                    for k in range(4):
                        nc.tensor.matmul(
                            ph0[:], w1sb[:, k, 0:128], rhs[k],
                            start=(k == 0), stop=(k == 3),
                        )
                    for k in range(4):
                        nc.tensor.matmul(
                            ph1[:], w1sb[:, k, 128:256], rhs[k],
                            start=(k == 0), stop=(k == 3),
                        )
                    for k in range(4):
                        nc.tensor.matmul(
                            pg[:], a1sb[:, k, :], rhs[k],
                            start=(k == 0), stop=(k == 3),
                        )
                    hr0 = hpool.tile([128, ESUB], bf16, tag="hr0")
                    nc.scalar.activation(hr0[:], ph0[:], Relu, bias=b1sb[:, 0:1])
                    hr1 = hpool.tile([128, ESUB], bf16, tag="hr1")
                    nc.scalar.activation(hr1[:], ph1[:], Relu, bias=b1sb[:, 1:2])
                    gr = hpool.tile([128, ESUB], bf16, tag="gr")
                    nc.scalar.activation(gr[:], pg[:], Relu, bias=a1bs[:, 0:1])
                    if STAGE <= 2:
                        nc.vector.tensor_copy(ast[:, sl], gr[0:64, :])
                        nc.vector.tensor_copy(zst[:, sl], hr0[0:1, :])
                        continue
                    pzt = pzp.tile([1, ESUB], f32, tag="pz")
                    nc.tensor.matmul(
                        pzt[:], w2sb[:, 0, :], hr0[:], start=True, stop=False
                    )
                    nc.tensor.matmul(
                        pzt[:], w2sb[:, 1, :], hr1[:], start=False, stop=True
                    )
                    nc.scalar.activation(
                        zst[:, sl], pzt[:], Ident, bias=b2sb[:, 0:1]
                    )
                    if STAGE <= 3:
                        nc.vector.tensor_copy(ast[:, sl], gr[0:64, :])
                        continue
                    pa = pzp.tile([64, ESUB], f32, tag="pa")
                    nc.tensor.matmul(pa[:], a2sb[:], gr[:], start=True, stop=True)
                    mt = hpool.tile([64, ESUB], f32, tag="mt")
                    nc.vector.tensor_scalar(
                        mt[:], gs[0:64, 2, sl], 1e9, a2bs[:, 0:1],
                        mybir.AluOpType.mult, mybir.AluOpType.add,
                    )
                    nc.vector.tensor_tensor(
                        out=ast[:, sl], in0=pa[:], in1=mt[:],
                        op=mybir.AluOpType.add,
                    )
                nc.sync.dma_start(
                    out=aout[:, J * NI : (J + 1) * NI], in_=ast[:]
                )
                nc.sync.dma_start(out=zout[J : J + 1, :], in_=zst[:])

    nc.compile()
    _split_multiwaits(nc)
    return nc


def _get_nc():
    if "nc" not in _NC_CACHE:
        _NC_CACHE["nc"] = _build_nc()
    return _NC_CACHE["nc"]


def _wrap_rows(table):
    """Place row n at (n+32768) % 65536 so sign-extended int16 indices hit
    the right row when the AP base is offset by +32768 rows."""
    n, w = table.shape
    out = np.zeros((WRAP, w), dtype=table.dtype)
    rows = (np.arange(n) + 32768) % WRAP
    out[rows] = table
    return out


def _idx_layout(idx):
    """int16 gather index layout: idx i -> partition i%16 (replicated in
    all 8 groups of 16 partitions), column i//16."""
    cols = idx.shape[0] // 16
    base = idx.astype(np.uint16).view(np.int16).reshape(cols, 16).T  # [16, cols]
    return np.tile(base, (8, 1)).copy()  # [128, cols]


def _prep_in_maps(inputs):
    emb = np.asarray(inputs["node_embeddings"], dtype=np.float32)
    W1 = np.asarray(inputs["W1"], dtype=np.float32)
    b1 = np.asarray(inputs["b1"], dtype=np.float32).reshape(-1)
    W2 = np.asarray(inputs["W2"], dtype=np.float32)
    b2 = np.asarray(inputs["b2"], dtype=np.float32).reshape(-1)
    A1 = np.asarray(inputs["A1"], dtype=np.float32)
    a1 = np.asarray(inputs["a1"], dtype=np.float32).reshape(-1)
    A2 = np.asarray(inputs["A2"], dtype=np.float32)
    a2 = np.asarray(inputs["a2"], dtype=np.float32).reshape(-1)
    edges = np.asarray(inputs["action_edges"])
    cnt = np.asarray(inputs["army_counts"]).astype(np.int64)
    max_send = int(np.asarray(inputs["max_army_send"]))
    assert max_send == A_DIM, max_send
    assert emb.shape == (N_NODES, D)
    assert edges.shape == (E_TOTAL, 2)

    emb_bf = emb.astype(BF16)
    # node-level army mask: 0 where army index < cnt-1 else -1
    mask = np.where(
        np.arange(A_DIM)[None, :] < (cnt - 1)[:, None], 0.0, -1.0
    ).astype(BF16)
    src_tab = np.concatenate(
        [emb_bf, mask, np.zeros((N_NODES, 64), BF16)], axis=1
    )  # [N, 384]
    wsrc = _wrap_rows(src_tab)
    wtgt = _wrap_rows(emb_bf)

    b1_r = b1.reshape(2, 128).T.astype(np.float32).copy()  # [128, 2]
    common = {
        "wsrc": wsrc,
        "wtgt": wtgt,
        "w1": W1.astype(BF16),
        "a1w": A1.astype(BF16),
        "a2w": A2.astype(BF16),
        "w2": W2.astype(BF16),
        "b1r": b1_r,
        "a1r": a1.reshape(128, 1).astype(np.float32),
        "b2r": b2.reshape(1, 1).astype(np.float32),
        "a2r": a2.reshape(64, 1).astype(np.float32),
    }
    nreal = min(EC, NBLK * EB)  # smaller only in debug (ATTACK_NBLK override)
    in_maps = []
    for c in range(N_CORES):
        s = edges[c * EC : c * EC + nreal, 0]
        t = edges[c * EC : c * EC + nreal, 1]
        sp = np.zeros((NBLK, NI), np.int64)
        tp = np.zeros((NBLK, NI), np.int64)
        sflat = np.zeros(NBLK * EB, np.int64)
        tflat = np.zeros(NBLK * EB, np.int64)
        sflat[:nreal] = s
        tflat[:nreal] = t
        sp[:, :EB] = sflat.reshape(NBLK, EB)
        tp[:, :EB] = tflat.reshape(NBLK, EB)
        in_maps.append(
            {
                **common,
                "sidx": _idx_layout(sp.reshape(-1)),
                "tidx": _idx_layout(tp.reshape(-1)),
            }
        )
    return in_maps


def _run(inputs, trace=False, trace_kwargs=None):
    nc = _get_nc()
    in_maps = _prep_in_maps(inputs)
    res = run_bass_kernel_spmd(
        nc,
        in_maps,
        list(range(N_CORES)),
        trace=trace,
        **(trace_kwargs or {}),
    )
    edge_logits = np.empty(E_TOTAL, np.float32)
    army_logits = np.empty((E_TOTAL, A_DIM), np.float32)
    for c in range(N_CORES):
        z = np.asarray(res.results[c]["zout"])[:, :EB].reshape(-1)[:EC]
        a = (
            np.asarray(res.results[c]["aout"])
            .reshape(64, NBLK, NI)[:, :, :EB]
            .reshape(64, NBLK * EB)[:, :EC]
            .T
        )
        edge_logits[c * EC : (c + 1) * EC] = z
        army_logits[c * EC : (c + 1) * EC] = a
    return (edge_logits, army_logits), res


def kernel(**inputs):
    (edge_logits, army_logits), _ = _run(inputs)
    return edge_logits, army_logits


# revision 13
# speedup vs baseline: 1.0530x; 1.0530x over previous
"""AttackHead kernel for 8 trn2 NeuronCores.

Strategy (edges sharded data-parallel across 8 cores, everything else
replicated — no collectives):
  - Host: cast node_embeddings to bf16 and build two "wrapped" gather
    tables (row (n+32768)%65536 holds node n, so int16 indices with
    sign-extension address all 50000 rows):
      * src table row: [emb(256) | army-mask(64) in {0,-1} | pad(64)] bf16
      * tgt table row: [emb(256)] bf16
  - Device, per 2048-edge block: two dma_gather(transpose=True) calls pull
    src/tgt rows FEATURE-MAJOR into SBUF ([128 part, elem/128, 2048]).
    Per 512-edge sub-chunk: PE does h = W1.T@e (8 mm), g = A1.T@e (4 mm),
    z = W2.T@relu(h) (2 mm), army = A2.T@relu(g) (1 mm), all bf16 inputs
    with f32 PSUM accumulation. ACT applies bias+relu. DVE applies
    army mask (gathered {0,-1} * 1e9 + a2) and stages outputs.
  - Outputs are written transposed ([64, E] army, row-chunked edge logits);
    host reassembles.
"""

import numpy as np
import ml_dtypes

import concourse.bass as bass
import concourse.bacc as bacc
import concourse.mybir as mybir
import concourse.tile as tile
from concourse.library_config import mlp
from concourse.bass_utils import run_bass_kernel_spmd

BF16 = ml_dtypes.bfloat16

N_CORES = 8
E_TOTAL = 500000
N_NODES = 50000
D = 256
A_DIM = 64
import os as _os

NI = int(_os.environ.get("ATTACK_NI", 1024))   # slots per gather block
STAGE = int(_os.environ.get("ATTACK_STAGE", 4))  # debug: 1=gather,2=+hg,3=+z,4=all
ESUB = 512             # edges per matmul sub-chunk
EC = E_TOTAL // N_CORES            # 62500 edges per core
# The SWDGE gather drops TRAILING int16-negative indices, so the last slot
# of every gather block must hold a small dummy index -> 511 real edges
# per 512-slot block.
EB = NI - 1                        # real edges per block
NBLK = int(_os.environ.get("ATTACK_NBLK", (EC + EB - 1) // EB))  # 123
EPAD = NBLK * NI                   # padded slot count
WRAP = 65536

_NC_CACHE = {}


def _split_multiwaits(nc):
    """Walrus build in this container only allows 1 sync wait per
    instruction (2 for EventSemaphore); split extras onto NoOps."""
    import bass_rust

    n_fixed = 0
    for fn in nc.m.functions:
        for b in fn.blocks:
            new_list = []
            changed = False
            for ins in b.instructions:
                si = ins.sync_info
                cap = 2 if isinstance(ins, mybir.InstEventSemaphore) else 1
                if si is not None and si.on_wait and len(si.on_wait) > cap:
                    waits = list(si.on_wait)
                    for j, w in enumerate(waits[:-1]):
                        nop = mybir.InstNoOp(
                            name=f"{ins.name}_waitfix{j}", ins=[], outs=[]
                        )
                        nop.engine = ins.engine
                        nop.sync_info = bass_rust.SyncInfo(
                            on_wait=[w], on_update=[]
                        )
                        new_list.append(nop)
                    si.on_wait = [waits[-1]]
                    ins.sync_info = si
                    changed = True
                    n_fixed += 1
                new_list.append(ins)
            if changed:
                b.instructions = new_list
    return n_fixed


def _build_nc():
    f32 = mybir.dt.float32
    bf16 = mybir.dt.bfloat16
    i16 = mybir.dt.int16
    Relu = mybir.ActivationFunctionType.Relu
    Ident = mybir.ActivationFunctionType.Identity

    nc = bacc.Bacc("TRN2", target_bir_lowering=False, debug=False)
    wsrc = nc.declare_dram_parameter("wsrc", [WRAP, 384], bf16, isOutput=False)
    wtgt = nc.declare_dram_parameter("wtgt", [WRAP, 256], bf16, isOutput=False)
    sidx = nc.declare_dram_parameter("sidx", [128, EPAD // 16], i16, isOutput=False)
    tidx = nc.declare_dram_parameter("tidx", [128, EPAD // 16], i16, isOutput=False)
    w1 = nc.declare_dram_parameter("w1", [512, 256], bf16, isOutput=False)
    a1w = nc.declare_dram_parameter("a1w", [512, 128], bf16, isOutput=False)
    a2w = nc.declare_dram_parameter("a2w", [128, 64], bf16, isOutput=False)
    w2 = nc.declare_dram_parameter("w2", [256, 1], bf16, isOutput=False)
    b1r = nc.declare_dram_parameter("b1r", [128, 2], f32, isOutput=False)
    a1r = nc.declare_dram_parameter("a1r", [128, 1], f32, isOutput=False)
    b2r = nc.declare_dram_parameter("b2r", [1, 1], f32, isOutput=False)
    a2r = nc.declare_dram_parameter("a2r", [64, 1], f32, isOutput=False)
    zout = nc.declare_dram_parameter("zout", [NBLK, NI], f32, isOutput=True)
    aout = nc.declare_dram_parameter("aout", [64, EPAD], f32, isOutput=True)

    with tile.TileContext(nc) as tc:
        nc.gpsimd.load_library(mlp)
        with (
            tc.tile_pool(name="const", bufs=1) as cpool,
            tc.tile_pool(name="gather", bufs=2) as gpool,
            tc.tile_pool(name="acts", bufs=3) as hpool,
            tc.tile_pool(name="stage", bufs=2) as spool,
            tc.tile_pool(name="psumHG", bufs=2, space="PSUM") as pp,
            tc.tile_pool(name="psumZA", bufs=1, space="PSUM") as pzp,
        ):
            w1sb = cpool.tile([128, 4, 256], bf16)
            nc.sync.dma_start(
                out=w1sb[:], in_=w1[:].rearrange("(k p) m -> p k m", p=128)
            )
            a1sb = cpool.tile([128, 4, 128], bf16)
            nc.sync.dma_start(
                out=a1sb[:], in_=a1w[:].rearrange("(k p) m -> p k m", p=128)
            )
            a2sb = cpool.tile([128, 64], bf16)
            nc.sync.dma_start(out=a2sb[:], in_=a2w[:])
            w2sb = cpool.tile([128, 2, 1], bf16)
            nc.sync.dma_start(
                out=w2sb[:], in_=w2[:].rearrange("(k p) m -> p k m", p=128)
            )
            b1sb = cpool.tile([128, 2], f32)
            nc.sync.dma_start(out=b1sb[:], in_=b1r[:])
            a1bs = cpool.tile([128, 1], f32)
            nc.sync.dma_start(out=a1bs[:], in_=a1r[:])
            b2sb = cpool.tile([1, 1], f32)
            nc.sync.dma_start(out=b2sb[:], in_=b2r[:])
            a2bs = cpool.tile([64, 1], f32)
            nc.sync.dma_start(out=a2bs[:], in_=a2r[:])
            sidx_sb = cpool.tile([128, EPAD // 16], i16)
            nc.sync.dma_start(out=sidx_sb[:], in_=sidx[:])
            tidx_sb = cpool.tile([128, EPAD // 16], i16)
            nc.sync.dma_start(out=tidx_sb[:], in_=tidx[:])

            IC = NI // 16  # idx columns per block
            for J in range(NBLK):
                gs = gpool.tile([128, 3, NI], bf16, tag="gs")
                nc.gpsimd.dma_gather(
                    out_ap=gs[:],
                    in_ap=wsrc[32768:, :],
                    idxs_ap=sidx_sb[:, J * IC : (J + 1) * IC],
                    num_idxs=NI,
                    num_idxs_reg=NI,
                    elem_size=384,
                    transpose=True,
                    single_packet=False,
                )
                gt = gpool.tile([128, 2, NI], bf16, tag="gt")
                nc.gpsimd.dma_gather(
                    out_ap=gt[:],
                    in_ap=wtgt[32768:, :],
                    idxs_ap=tidx_sb[:, J * IC : (J + 1) * IC],
                    num_idxs=NI,
                    num_idxs_reg=NI,
                    elem_size=256,
                    transpose=True,
                    single_packet=False,
                )
                ast = spool.tile([64, NI], f32, tag="ast")
                zst = spool.tile([1, NI], f32, tag="zst")
                if STAGE <= 1:
                    # debug: dump gathered src chunk0 into aout rows
                    nc.vector.tensor_copy(ast[:], gs[0:64, 0, :])
                    nc.vector.tensor_copy(zst[:], gt[0:1, 0, :])
                    nc.sync.dma_start(
                        out=aout[:, J * NI : (J + 1) * NI], in_=ast[:]
                    )
                    nc.sync.dma_start(out=zout[J : J + 1, :], in_=zst[:])
                    continue
                for s in range(NI // ESUB):
                    sl = slice(s * ESUB, (s + 1) * ESUB)
                    ph0 = pp.tile([128, ESUB], f32, tag="ph0")
                    ph1 = pp.tile([128, ESUB], f32, tag="ph1")
                    pg = pp.tile([128, ESUB], f32, tag="pg")
                    rhs = [gs[:, 0, sl], gs[:, 1, sl], gt[:, 0, sl], gt[:, 1, sl]]
                    for k in range(4):
                        nc.tensor.matmul(
                            ph0[:], w1sb[:, k, 0:128], rhs[k],
                            start=(k == 0), stop=(k == 3),
                        )
                    for k in range(4):
                        nc.tensor.matmul(
                            ph1[:], w1sb[:, k, 128:256], rhs[k],
                            start=(k == 0), stop=(k == 3),
                        )
                    for k in range(4):
                        nc.tensor.matmul(
                            pg[:], a1sb[:, k, :], rhs[k],
                            start=(k == 0), stop=(k == 3),
                        )
                    hr0 = hpool.tile([128, ESUB], bf16, tag="hr0")
                    nc.scalar.activation(hr0[:], ph0[:], Relu, bias=b1sb[:, 0:1])
                    hr1 = hpool.tile([128, ESUB], bf16, tag="hr1")
                    nc.scalar.activation(hr1[:], ph1[:], Relu, bias=b1sb[:, 1:2])
                    gr = hpool.tile([128, ESUB], bf16, tag="gr")
                    nc.scalar.activation(gr[:], pg[:], Relu, bias=a1bs[:, 0:1])
                    if STAGE <= 2:
                        nc.vector.tensor_copy(ast[:, sl], gr[0:64, :])
                        nc.vector.tensor_copy(zst[:, sl], hr0[0:1, :])
                        continue
                    pzt = pzp.tile([1, ESUB], f32, tag="pz")
                    nc.tensor.matmul(
                        pzt[:], w2sb[:, 0, :], hr0[:], start=True, stop=False
                    )
                    nc.tensor.matmul(
                        pzt[:], w2sb[:, 1, :], hr1[:], start=False, stop=True
                    )
                    nc.scalar.activation(
                        zst[:, sl], pzt[:], Ident, bias=b2sb[:, 0:1]
                    )
                    if STAGE <= 3:
                        nc.vector.tensor_copy(ast[:, sl], gr[0:64, :])
                        continue
                    pa = pzp.tile([64, ESUB], f32, tag="pa")
                    nc.tensor.matmul(pa[:], a2sb[:], gr[:], start=True, stop=True)
                    mt = hpool.tile([64, ESUB], f32, tag="mt")
                    nc.vector.tensor_scalar(
                        mt[:], gs[0:64, 2, sl], 1e9, a2bs[:, 0:1],
                        mybir.AluOpType.mult, mybir.AluOpType.add,
                    )
                    nc.vector.tensor_tensor(
                        out=ast[:, sl], in0=pa[:], in1=mt[:],
                        op=mybir.AluOpType.add,
                    )
                nc.sync.dma_start(
                    out=aout[:, J * NI : (J + 1) * NI], in_=ast[:]
                )
                nc.sync.dma_start(out=zout[J : J + 1, :], in_=zst[:])

    nc.compile()
    _split_multiwaits(nc)
    return nc


def _get_nc():
    if "nc" not in _NC_CACHE:
        _NC_CACHE["nc"] = _build_nc()
    return _NC_CACHE["nc"]


def _wrap_rows(table):
    """Place row n at (n+32768) % 65536 so sign-extended int16 indices hit
    the right row when the AP base is offset by +32768 rows."""
    n, w = table.shape
    out = np.zeros((WRAP, w), dtype=table.dtype)
    rows = (np.arange(n) + 32768) % WRAP
    out[rows] = table
    return out


def _idx_layout(idx):
    """int16 gather index layout: idx i -> partition i%16 (replicated in
    all 8 groups of 16 partitions), column i//16."""
    cols = idx.shape[0] // 16
    base = idx.astype(np.uint16).view(np.int16).reshape(cols, 16).T  # [16, cols]
    return np.tile(base, (8, 1)).copy()  # [128, cols]


def _prep_in_maps(inputs):
    emb = np.asarray(inputs["node_embeddings"], dtype=np.float32)
    W1 = np.asarray(inputs["W1"], dtype=np.float32)
    b1 = np.asarray(inputs["b1"], dtype=np.float32).reshape(-1)
    W2 = np.asarray(inputs["W2"], dtype=np.float32)
    b2 = np.asarray(inputs["b2"], dtype=np.float32).reshape(-1)
    A1 = np.asarray(inputs["A1"], dtype=np.float32)
    a1 = np.asarray(inputs["a1"], dtype=np.float32).reshape(-1)
    A2 = np.asarray(inputs["A2"], dtype=np.float32)
    a2 = np.asarray(inputs["a2"], dtype=np.float32).reshape(-1)
    edges = np.asarray(inputs["action_edges"])
    cnt = np.asarray(inputs["army_counts"]).astype(np.int64)
    max_send = int(np.asarray(inputs["max_army_send"]))
    assert max_send == A_DIM, max_send
    assert emb.shape == (N_NODES, D)
    assert edges.shape == (E_TOTAL, 2)

    emb_bf = emb.astype(BF16)
    # node-level army mask: 0 where army index < cnt-1 else -1
    mask = np.where(
        np.arange(A_DIM)[None, :] < (cnt - 1)[:, None], 0.0, -1.0
    ).astype(BF16)
    src_tab = np.concatenate(
        [emb_bf, mask, np.zeros((N_NODES, 64), BF16)], axis=1
    )  # [N, 384]
    wsrc = _wrap_rows(src_tab)
    wtgt = _wrap_rows(emb_bf)

    b1_r = b1.reshape(2, 128).T.astype(np.float32).copy()  # [128, 2]
    common = {
        "wsrc": wsrc,
        "wtgt": wtgt,
        "w1": W1.astype(BF16),
        "a1w": A1.astype(BF16),
        "a2w": A2.astype(BF16),
        "w2": W2.astype(BF16),
        "b1r": b1_r,
        "a1r": a1.reshape(128, 1).astype(np.float32),
        "b2r": b2.reshape(1, 1).astype(np.float32),
        "a2r": a2.reshape(64, 1).astype(np.float32),
    }
    nreal = min(EC, NBLK * EB)  # smaller only in debug (ATTACK_NBLK override)
    in_maps = []
    for c in range(N_CORES):
        s = edges[c * EC : c * EC + nreal, 0]
        t = edges[c * EC : c * EC + nreal, 1]
        sp = np.zeros((NBLK, NI), np.int64)
        tp = np.zeros((NBLK, NI), np.int64)
        sflat = np.zeros(NBLK * EB, np.int64)
        tflat = np.zeros(NBLK * EB, np.int64)
        sflat[:nreal] = s
        tflat[:nreal] = t
        sp[:, :EB] = sflat.reshape(NBLK, EB)
        tp[:, :EB] = tflat.reshape(NBLK, EB)
        in_maps.append(
            {
                **common,
                "sidx": _idx_layout(sp.reshape(-1)),
                "tidx": _idx_layout(tp.reshape(-1)),
            }
        )
    return in_maps


def _run(inputs, trace=False, trace_kwargs=None):
    nc = _get_nc()
    in_maps = _prep_in_maps(inputs)
    res = run_bass_kernel_spmd(
        nc,
        in_maps,
        list(range(N_CORES)),
        trace=trace,
        **(trace_kwargs or {}),
    )
    edge_logits = np.empty(E_TOTAL, np.float32)
    army_logits = np.empty((E_TOTAL, A_DIM), np.float32)
    for c in range(N_CORES):
        z = np.asarray(res.results[c]["zout"])[:, :EB].reshape(-1)[:EC]
        a = (
            np.asarray(res.results[c]["aout"])
            .reshape(64, NBLK, NI)[:, :, :EB]
            .reshape(64, NBLK * EB)[:, :EC]
            .T
        )
        edge_logits[c * EC : (c + 1) * EC] = z
        army_logits[c * EC : (c + 1) * EC] = a
    return (edge_logits, army_logits), res


def kernel(**inputs):
    (edge_logits, army_logits), _ = _run(inputs)
    return edge_logits, army_logits


# revision 17
# speedup vs baseline: 1.6001x; 1.5196x over previous
"""AttackHead kernel v2: src-locality gathers.

v1 was GpSimd-bound: SWDGE gather descriptor generation costs ~8.5ns/index
and v1 needed 2 indices per edge. v2 shards edges by SRC NODE RANGE (so each
core's src values span N/8=6250 nodes, ~10 edges per node), sorts by src,
and per 1024-slot window gathers each unique src row ONCE (<=127 uniques,
128-index dma_gather). The unique rows are pushed through W1/A1 on the PE at
unique granularity, then expanded to edge granularity with a host-provided
one-hot selection matrix S via matmuls that accumulate directly into the
same PSUM banks as the per-edge tgt-side matmuls. Tgt side stays per-edge
(2048-index gathers). Index count per edge drops from 2 to ~1.06.

Outputs are computed in sorted order; the host scatters them back via the
original edge ids.
"""

import numpy as np
import ml_dtypes

import concourse.bass as bass
import concourse.bacc as bacc
import concourse.mybir as mybir
import concourse.tile as tile
from concourse.library_config import mlp
from concourse.bass_utils import run_bass_kernel_spmd

BF16 = ml_dtypes.bfloat16

N_CORES = 8
E_TOTAL = 500000
N_NODES = 50000
D = 256
A_DIM = 64
NSHARD = N_NODES // N_CORES        # 6250 nodes per core's src range

import os as _os

NBLK = int(_os.environ.get("ATTACK2_NBLK", 32))  # 2048-slot blocks per core
BLK = 2048
WIN = 1024                          # slots per unique-window
NWIN = NBLK * BLK // WIN            # 64
WCAP = WIN - 1                      # real edges per window (last slot dummy)
UMAX = 127                          # real uniques per window (slot 127 dummy)
SLOTS = NBLK * BLK                  # 65536
NSUB = SLOTS // 512                 # 128 sub-chunks
WRAP = 65536

_NC_CACHE = {}


def _split_multiwaits(nc):
    import bass_rust

    n_fixed = 0
    for fn in nc.m.functions:
        for b in fn.blocks:
            new_list = []
            changed = False
            for ins in b.instructions:
                si = ins.sync_info
                cap = 2 if isinstance(ins, mybir.InstEventSemaphore) else 1
                if si is not None and si.on_wait and len(si.on_wait) > cap:
                    waits = list(si.on_wait)
                    for j, w in enumerate(waits[:-1]):
                        nop = mybir.InstNoOp(
                            name=f"{ins.name}_waitfix{j}", ins=[], outs=[]
                        )
                        nop.engine = ins.engine
                        nop.sync_info = bass_rust.SyncInfo(
                            on_wait=[w], on_update=[]
                        )
                        new_list.append(nop)
                    si.on_wait = [waits[-1]]
                    ins.sync_info = si
                    changed = True
                    n_fixed += 1
                new_list.append(ins)
            if changed:
                b.instructions = new_list
    return n_fixed


def _build_nc():
    f32 = mybir.dt.float32
    bf16 = mybir.dt.bfloat16
    i16 = mybir.dt.int16
    Relu = mybir.ActivationFunctionType.Relu
    Ident = mybir.ActivationFunctionType.Identity
    Copy = mybir.ActivationFunctionType.Copy

    nc = bacc.Bacc("TRN2", target_bir_lowering=False, debug=False)
    wsrc = nc.declare_dram_parameter("wsrc", [WRAP, 384], bf16, isOutput=False)
    wtgt = nc.declare_dram_parameter("wtgt", [WRAP, 256], bf16, isOutput=False)
    uidx = nc.declare_dram_parameter("uidx", [128, NWIN * 8], i16, isOutput=False)
    tidx = nc.declare_dram_parameter("tidx", [128, SLOTS // 16], i16, isOutput=False)
    sdram = nc.declare_dram_parameter("sdram", [128, NSUB * 512], bf16, isOutput=False)
    wall = nc.declare_dram_parameter("wall", [256, 384], bf16, isOutput=False)
    w1t = nc.declare_dram_parameter("w1t", [256, 256], bf16, isOutput=False)
    a1t = nc.declare_dram_parameter("a1t", [256, 128], bf16, isOutput=False)
    a2w = nc.declare_dram_parameter("a2w", [128, 64], bf16, isOutput=False)
    w2 = nc.declare_dram_parameter("w2", [256, 1], bf16, isOutput=False)
    idn = nc.declare_dram_parameter("idn", [128, 128], bf16, isOutput=False)
    b1r = nc.declare_dram_parameter("b1r", [128, 2], f32, isOutput=False)
    a1r = nc.declare_dram_parameter("a1r", [128, 1], f32, isOutput=False)
    b2r = nc.declare_dram_parameter("b2r", [1, 1], f32, isOutput=False)
    a2r = nc.declare_dram_parameter("a2r", [64, 1], f32, isOutput=False)
    zout = nc.declare_dram_parameter("zout", [NBLK, BLK], f32, isOutput=True)
    aout = nc.declare_dram_parameter("aout", [64, SLOTS], f32, isOutput=True)

    with tile.TileContext(nc) as tc:
        nc.gpsimd.load_library(mlp)
        with (
            tc.tile_pool(name="const", bufs=1) as cpool,
            tc.tile_pool(name="gather", bufs=4) as gpool,
            tc.tile_pool(name="acts", bufs=4) as hpool,
            tc.tile_pool(name="stage", bufs=3) as spool,
            tc.tile_pool(name="psum", bufs=1, space="PSUM") as pp,
        ):
            wall_sb = cpool.tile([128, 2, 384], bf16)
            nc.sync.dma_start(
                out=wall_sb[:], in_=wall[:].rearrange("(k p) m -> p k m", p=128)
            )
            w1t_sb = cpool.tile([128, 2, 256], bf16)
            nc.sync.dma_start(
                out=w1t_sb[:], in_=w1t[:].rearrange("(k p) m -> p k m", p=128)
            )
            a1t_sb = cpool.tile([128, 2, 128], bf16)
            nc.sync.dma_start(
                out=a1t_sb[:], in_=a1t[:].rearrange("(k p) m -> p k m", p=128)
            )
            a2sb = cpool.tile([128, 64], bf16)
            nc.sync.dma_start(out=a2sb[:], in_=a2w[:])
            w2sb = cpool.tile([128, 2, 1], bf16)
            nc.sync.dma_start(
                out=w2sb[:], in_=w2[:].rearrange("(k p) m -> p k m", p=128)
            )
            ident = cpool.tile([128, 128], bf16)
            nc.sync.dma_start(out=ident[:], in_=idn[:])
            b1sb = cpool.tile([128, 2], f32)
            nc.sync.dma_start(out=b1sb[:], in_=b1r[:])
            a1bs = cpool.tile([128, 1], f32)
            nc.sync.dma_start(out=a1bs[:], in_=a1r[:])
            b2sb = cpool.tile([1, 1], f32)
            nc.sync.dma_start(out=b2sb[:], in_=b2r[:])
            a2bs = cpool.tile([64, 1], f32)
            nc.sync.dma_start(out=a2bs[:], in_=a2r[:])
            uidx_sb = cpool.tile([128, NWIN * 8], i16)
            nc.sync.dma_start(out=uidx_sb[:], in_=uidx[:])
            tidx_sb = cpool.tile([128, SLOTS // 16], i16)
            nc.sync.dma_start(out=tidx_sb[:], in_=tidx[:])

            for J in range(NBLK):
                gt = gpool.tile([128, 2, BLK], bf16, tag="gt")
                nc.gpsimd.dma_gather(
                    out_ap=gt[:],
                    in_ap=wtgt[32768:, :],
                    idxs_ap=tidx_sb[:, J * 128 : (J + 1) * 128],
                    num_idxs=BLK,
                    num_idxs_reg=BLK,
                    elem_size=256,
                    transpose=True,
                    single_packet=False,
                )
                ast = spool.tile([64, BLK], f32, tag="ast")
                zst = spool.tile([1, BLK], f32, tag="zst")
                for h in range(2):
                    w = 2 * J + h
                    gw = gpool.tile([128, 3, 128], bf16, tag="gw")
                    nc.gpsimd.dma_gather(
                        out_ap=gw[:],
                        in_ap=wsrc[32768:, :],
                        idxs_ap=uidx_sb[:, w * 8 : (w + 1) * 8],
                        num_idxs=128,
                        num_idxs_reg=128,
                        elem_size=384,
                        transpose=True,
                        single_packet=False,
                    )
                    # X^T[u, 0:384] = (emb_u @ Wall) ; [u, 384:448] = mask_u
                    xt = pp.tile([128, 384], mybir.dt.float32, tag="xt")
                    nc.tensor.matmul(
                        xt[:, 0:384], gw[:, 0, :], wall_sb[:, 0, :],
                        start=True, stop=False,
                    )
                    nc.tensor.matmul(
                        xt[:, 0:384], gw[:, 1, :], wall_sb[:, 1, :],
                        start=False, stop=True,
                    )
                    xm = pp.tile([128, 64], bf16, tag="xm")
                    nc.tensor.transpose(
                        xm[:], gw[0:64, 2, :], ident[0:64, 0:64]
                    )
                    xts = hpool.tile([128, 448], bf16, tag="xts")
                    nc.scalar.activation(xts[:, 0:384], xt[:, 0:384], Copy)
                    nc.scalar.activation(xts[:, 384:448], xm[:], Copy)
                    for s in range(2):
                        j = w * 2 + s                 # global sub index
                        lo = h * WIN + s * 512        # block-local edge offset
                        sl = slice(lo, lo + 512)
                        St = hpool.tile([128, 512], bf16, tag="S")
                        nc.sync.dma_start(
                            out=St[:], in_=sdram[:, j * 512 : (j + 1) * 512]
                        )
                        ph0 = pp.tile([128, 512], mybir.dt.float32, tag="ph0")
                        ph1 = pp.tile([128, 512], mybir.dt.float32, tag="ph1")
                        pg = pp.tile([128, 512], mybir.dt.float32, tag="pg")
                        pm = pp.tile([64, 512], mybir.dt.float32, tag="pm")
                        nc.tensor.matmul(
                            ph0[:], xts[:, 0:128], St[:], start=True, stop=False
                        )
                        nc.tensor.matmul(
                            ph0[:], w1t_sb[:, 0, 0:128], gt[:, 0, sl],
                            start=False, stop=False,
                        )
                        nc.tensor.matmul(
                            ph0[:], w1t_sb[:, 1, 0:128], gt[:, 1, sl],
                            start=False, stop=True,
                        )
                        nc.tensor.matmul(
                            ph1[:], xts[:, 128:256], St[:], start=True, stop=False
                        )
                        nc.tensor.matmul(
                            ph1[:], w1t_sb[:, 0, 128:256], gt[:, 0, sl],
                            start=False, stop=False,
                        )
                        nc.tensor.matmul(
                            ph1[:], w1t_sb[:, 1, 128:256], gt[:, 1, sl],
                            start=False, stop=True,
                        )
                        nc.tensor.matmul(
                            pg[:], xts[:, 256:384], St[:], start=True, stop=False
                        )
                        nc.tensor.matmul(
                            pg[:], a1t_sb[:, 0, :], gt[:, 0, sl],
                            start=False, stop=False,
                        )
                        nc.tensor.matmul(
                            pg[:], a1t_sb[:, 1, :], gt[:, 1, sl],
                            start=False, stop=True,
                        )
                        nc.tensor.matmul(
                            pm[:], xts[:, 384:448], St[:], start=True, stop=True
                        )
                        hr0 = hpool.tile([128, 512], bf16, tag="hr0")
                        nc.scalar.activation(hr0[:], ph0[:], Relu, bias=b1sb[:, 0:1])
                        hr1 = hpool.tile([128, 512], bf16, tag="hr1")
                        nc.scalar.activation(hr1[:], ph1[:], Relu, bias=b1sb[:, 1:2])
                        gr = hpool.tile([128, 512], bf16, tag="gr")
                        nc.scalar.activation(gr[:], pg[:], Relu, bias=a1bs[:, 0:1])
                        pz = pp.tile([1, 512], mybir.dt.float32, tag="pz")
                        nc.tensor.matmul(
                            pz[:], w2sb[:, 0, :], hr0[:], start=True, stop=False
                        )
                        nc.tensor.matmul(
                            pz[:], w2sb[:, 1, :], hr1[:], start=False, stop=True
                        )
                        pa = pp.tile([64, 512], mybir.dt.float32, tag="pa")
                        nc.tensor.matmul(pa[:], a2sb[:], gr[:], start=True, stop=True)
                        nc.scalar.activation(
                            zst[:, sl], pz[:], Ident, bias=b2sb[:, 0:1]
                        )
                        mt = hpool.tile([64, 512], mybir.dt.float32, tag="mt")
                        nc.vector.tensor_scalar(
                            mt[:], pm[:], 1e9, a2bs[:, 0:1],
                            mybir.AluOpType.mult, mybir.AluOpType.add,
                        )
                        nc.vector.tensor_tensor(
                            out=ast[:, sl], in0=pa[:], in1=mt[:],
                            op=mybir.AluOpType.add,
                        )
                nc.sync.dma_start(out=aout[:, J * BLK : (J + 1) * BLK], in_=ast[:])
                nc.sync.dma_start(out=zout[J : J + 1, :], in_=zst[:])

    nc.compile()
    _split_multiwaits(nc)
    return nc


def _get_nc():
    if "nc" not in _NC_CACHE:
        _NC_CACHE["nc"] = _build_nc()
    return _NC_CACHE["nc"]


def _wrap_rows(table):
    n, w = table.shape
    out = np.zeros((WRAP, w), dtype=table.dtype)
    out[(np.arange(n) + 32768) % WRAP] = table
    return out


def _wrap_i16(idx):
    return idx.astype(np.uint16).view(np.int16)


def _idx_layout(idx):
    """idx i -> [i%16 (+16k replicas), i//16]"""
    cols = idx.shape[0] // 16
    base = _wrap_i16(idx).reshape(cols, 16).T
    return np.tile(base, (8, 1)).copy()


def _pack_core(src, tgt, gids):
    """Sort by src and pack into windows with <=UMAX uniques and <=WCAP edges.
    Returns (uniq [NWIN,128] int64, slotid [SLOTS] int64, tgt_slots [SLOTS],
    orig [SLOTS] int64 (-1 = dummy))."""
    order = np.argsort(src, kind="stable")
    s, t, g = src[order], tgt[order], gids[order]
    L = len(s)
    uniq = np.zeros((NWIN, 128), np.int64)
    slotid = np.zeros(SLOTS, np.int64)
    tgt_slots = np.zeros(SLOTS, np.int64)
    orig = np.full(SLOTS, -1, np.int64)
    pos = 0
    new = np.empty(L, bool)
    if L:
        new[0] = True
        new[1:] = s[1:] != s[:-1]
    for w in range(NWIN):
        if pos >= L:
            continue
        span = min(WCAP, L - pos)
        nw = new[pos : pos + span].copy()
        nw[0] = True
        cu = np.cumsum(nw)
        over = np.nonzero(cu > UMAX)[0]
        take = int(over[0]) if len(over) else span
        cu = cu[:take]
        base = w * WIN
        sl = slice(base, base + take)
        slotid[sl] = cu - 1
        tgt_slots[sl] = t[pos : pos + take]
        orig[sl] = g[pos : pos + take]
        u = s[pos : pos + take][nw[:take]]
        uniq[w, : len(u)] = u
        pos += take
    if pos < L:
        raise RuntimeError(f"packing overflow: {L - pos} edges left")
    return uniq, slotid, tgt_slots, orig


def _prep_in_maps(inputs):
    emb = np.asarray(inputs["node_embeddings"], dtype=np.float32)
    W1 = np.asarray(inputs["W1"], dtype=np.float32)
    b1 = np.asarray(inputs["b1"], dtype=np.float32).reshape(-1)
    W2 = np.asarray(inputs["W2"], dtype=np.float32)
    b2 = np.asarray(inputs["b2"], dtype=np.float32).reshape(-1)
    A1 = np.asarray(inputs["A1"], dtype=np.float32)
    a1 = np.asarray(inputs["a1"], dtype=np.float32).reshape(-1)
    A2 = np.asarray(inputs["A2"], dtype=np.float32)
    a2 = np.asarray(inputs["a2"], dtype=np.float32).reshape(-1)
    edges = np.asarray(inputs["action_edges"])
    cnt = np.asarray(inputs["army_counts"]).astype(np.int64)
    max_send = int(np.asarray(inputs["max_army_send"]))
    assert max_send == A_DIM and emb.shape == (N_NODES, D)

    emb_bf = emb.astype(BF16)
    mask = np.where(
        np.arange(A_DIM)[None, :] < (cnt - 1)[:, None], 0.0, -1.0
    ).astype(BF16)
    wsrc = _wrap_rows(
        np.concatenate([emb_bf, mask, np.zeros((N_NODES, 64), BF16)], axis=1)
    )
    wtgt = _wrap_rows(emb_bf)
    common = {
        "wsrc": wsrc,
        "wtgt": wtgt,
        "wall": np.concatenate([W1[:256], A1[:256]], axis=1).astype(BF16),
        "w1t": W1[256:].astype(BF16),
        "a1t": A1[256:].astype(BF16),
        "a2w": A2.astype(BF16),
        "w2": W2.astype(BF16),
        "idn": np.eye(128, dtype=BF16),
        "b1r": b1.reshape(2, 128).T.astype(np.float32).copy(),
        "a1r": a1.reshape(128, 1).astype(np.float32),
        "b2r": b2.reshape(1, 1).astype(np.float32),
        "a2r": a2.reshape(64, 1).astype(np.float32),
    }
    src_all = edges[:, 0].astype(np.int64)
    tgt_all = edges[:, 1].astype(np.int64)
    shard = np.clip(src_all // NSHARD, 0, N_CORES - 1)
    in_maps = []
    origs = []
    eye = np.eye(128, dtype=BF16)
    for c in range(N_CORES):
        gids = np.nonzero(shard == c)[0]
        if NBLK < 32:  # debug-size build: keep a src sub-range that fits
            hi = c * NSHARD + max(1, NSHARD * NBLK // 36)
            gids = gids[src_all[gids] < hi]
        uniq, slotid, tgt_slots, orig = _pack_core(
            src_all[gids], tgt_all[gids], gids
        )
        S = eye[:, slotid]  # [128, SLOTS] one-hot of slot ids
        S = np.ascontiguousarray(
            S.reshape(128, NSUB, 512).reshape(128, NSUB * 512)
        )
        in_maps.append(
            {
                **common,
                "uidx": np.tile(
                    _wrap_i16(uniq.reshape(NWIN, 8, 16))
                    .transpose(2, 0, 1)
                    .reshape(16, NWIN * 8),
                    (8, 1),
                ).copy(),
                "tidx": _idx_layout(tgt_slots),
                "sdram": S,
            }
        )
        origs.append(orig)
    return in_maps, origs


def _run(inputs, trace=False, trace_kwargs=None):
    nc = _get_nc()
    in_maps, origs = _prep_in_maps(inputs)
    res = run_bass_kernel_spmd(
        nc, in_maps, list(range(N_CORES)), trace=trace, **(trace_kwargs or {})
    )
    edge_logits = np.empty(E_TOTAL, np.float32)
    army_logits = np.empty((E_TOTAL, A_DIM), np.float32)
    for c in range(N_CORES):
        z = np.asarray(res.results[c]["zout"]).reshape(-1)
        a = np.asarray(res.results[c]["aout"])
        orig = origs[c]
        valid = orig >= 0
        edge_logits[orig[valid]] = z[valid]
        army_logits[orig[valid]] = a[:, valid].T
    return (edge_logits, army_logits), res


def kernel(**inputs):
    (edge_logits, army_logits), _ = _run(inputs)
    return edge_logits, army_logits
